# revision 1
# baseline (speedup 1.0000x reference)
"""DirectPathAttenuationGNN Trainium2 kernel.

Strategy: data-parallel over graphs (512 graphs per core x 8 cores). The
graph topology is the fixed complete graph K9 (9 sensors, 72 directed
edges), so all gathers/scatters are per-graph-local and are expressed as
contiguous-slice / broadcast access patterns fed directly to the tensor
engine. Activations live transposed [H=128 partitions, tokens] in SBUF for
the whole network; only phys features stream in and per-edge logits stream
out. Matmuls run in float32r mode (1 cycle/row at N>=256).

Host side: phys edge-feature computation, weight folding (mean-aggregation
folded into node weights since deg==8), final sigmoid + pair-mean.
"""

import sys

if "/opt/trn_rl_repo" not in sys.path:
    sys.path.insert(0, "/opt/trn_rl_repo")

import numpy as np

B = 4096
S = 9
EPG = 72          # directed edges per graph
H = 128
L = 4
NCORES = 8
GC = B // NCORES  # graphs per core = 512
G = 256           # graphs per block
NBLK = GC // G    # 2
ET = EPG * G      # edge tokens per block = 18432
NT = S * G        # node tokens per block = 2304
TS = 512          # tile size (psum bank, fp32)
NTILE = ET // TS  # 36 edge tiles per block
EPS = np.float32(1e-8)

_prog_cache = {}


# ---------------------------------------------------------------------------
# host-side helpers
# ---------------------------------------------------------------------------

def _edge_struct():
    r_idx = np.repeat(np.arange(S), 8)              # [72] src node of edge e
    k_idx = np.tile(np.arange(8), S)
    c_idx = (r_idx + 1 + k_idx) % S                 # [72] dst node of edge e
    return r_idx, c_idx


def _build_phys(x_nodes, damage_locs):
    """phys [B, 72, 6] float32, device edge order, exact reference formulas."""
    xg = x_nodes.reshape(B, S, 2)
    r_idx, c_idx = _edge_struct()
    src = xg[:, r_idx, :]                           # [B,72,2]
    dst = xg[:, c_idx, :]
    dmg = damage_locs[:, None, :]                   # [B,1,2]

    vec = src - dst
    edge_len = np.sqrt(np.sum(vec * vec, -1) + EPS)
    d21 = dst - src
    l2 = np.clip(np.sum(d21 * d21, -1), EPS, None)
    t = np.clip(np.sum((dmg - src) * d21, -1) / l2, np.float32(0.0), np.float32(1.0))
    proj = src + t[..., None] * d21
    d_path = np.sqrt(np.sum((dmg - proj) ** 2, -1) + EPS)
    d_tx = np.sqrt(np.sum((src - dmg) ** 2, -1) + EPS)
    d_rx = np.sqrt(np.sum((dst - dmg) ** 2, -1) + EPS)
    phys = np.stack(
        [vec[..., 0], vec[..., 1], edge_len, d_path, d_tx, d_rx], axis=-1
    )
    return np.ascontiguousarray(phys.astype(np.float32))


# ---------------------------------------------------------------------------
# device program
# ---------------------------------------------------------------------------

def _build_program():
    from concourse import bacc, mybir, tile
    from contextlib import ExitStack

    f32 = mybir.dt.float32
    f32r = mybir.dt.float32r
    AF = mybir.ActivationFunctionType
    ALU = mybir.AluOpType

    nc = bacc.Bacc("TRN2", target_bir_lowering=False, debug=False)

    # ---- dram I/O
    xT_d = nc.dram_tensor("xT", [2, NBLK * NT], f32r, kind="ExternalInput")
    # phys features packed 4-up along partitions: rows 32q+f hold feature f of
    # edge tile 4m+q (for the row-packed K=6 encoder matmuls)
    physT_d = nc.dram_tensor("physT", [H, NBLK * ET // 4], f32r, kind="ExternalInput")
    # packed weights: per layer [w1c | w1a | w1b | w2 | wna | wnb | wn2]
    wl_d = nc.dram_tensor("wl", [H, L * 7 * H], f32r, kind="ExternalInput")
    # [encew2 | ident | decw1 | decw2b]
    wbig_d = nc.dram_tensor("wbig", [H, 2 * H + 64 + 2 + 64], f32r, kind="ExternalInput")
    # [encew1 replicated at partition bases 0/32/64/96 | encnw (2 rows)]
    encsm_d = nc.dram_tensor("encsm", [H, 2 * H], f32r, kind="ExternalInput")
    # biases: eb1[0:4] eb2[4:8] nb1[8:12] nb2[12:16] encnb[16] enceb1[17]
    #         enceb2[18] decb1x2[19]
    bp_d = nc.dram_tensor("bp", [H, 20], f32, kind="ExternalInput")
    z2_d = nc.dram_tensor("z2", [1, NBLK * ET], f32, kind="ExternalOutput")

    GSZ = 3                      # edge tiles per emission group
    NGRP = NTILE // GSZ          # 12

    with tile.TileContext(nc) as tc:
        with ExitStack() as ctx:
            wpool = ctx.enter_context(tc.tile_pool(name="w", bufs=1))
            sb = ctx.enter_context(tc.tile_pool(name="sb", bufs=1))
            ps = ctx.enter_context(tc.tile_pool(name="ps", bufs=1, space="PSUM"))

            # DMA order matters: encoder inputs first so compute starts
            # immediately; per-layer weight packs are emitted lazily at first
            # use so they queue behind only what precedes them.
            encsm = wpool.tile([H, 2 * H], f32r, name="encsm", tag="encsm")
            nc.sync.dma_start(encsm[:], encsm_d.ap())

            bp = wpool.tile([H, 20], f32, name="bp", tag="bp")
            nc.sync.dma_start(bp[:], bp_d.ap())
            wbig = wpool.tile([H, 2 * H + 64 + 2 + 64], f32r, name="wbig", tag="wbig")
            nc.sync.dma_start(wbig[:], wbig_d.ap())
            _prefetch_wl0 = True  # layer-0 weights queued right behind wbig

            encnw = encsm[0:2, H:2 * H]
            encew2 = wbig[:, 0:H]
            ident = wbig[:, H:2 * H]
            decw1 = wbig[:, 2 * H:2 * H + 64]
            decw2b = wbig[:, 2 * H + 64:2 * H + 66]
            wg = wbig[:, 2 * H + 66:2 * H + 130]
            eb1 = bp[:, 0:L]
            eb2 = bp[:, L:2 * L]
            nb1 = bp[:, 2 * L:3 * L]
            nb2 = bp[:, 3 * L:4 * L]
            encnb = bp[:, 16:17]
            enceb1 = bp[:, 17:18]
            enceb2 = bp[:, 18:19]
            decb1x2 = bp[:, 19:20]

            wl_tiles = {}

            def get_wl(l):
                """Layer-l packed weights, DMA'd on first use."""
                if l not in wl_tiles:
                    t = wpool.tile([H, 7 * H], f32r, name=f"wl{l}", tag=f"wl{l}")
                    nc.sync.dma_start(t[:], wl_d.ap()[:, l * 7 * H:(l + 1) * 7 * H])
                    wl_tiles[l] = t
                return wl_tiles[l]

            def wsl(l, k):
                return get_wl(l)[:, k * H:(k + 1) * H]
            # slice order: w1c=0, w1a=1, w1b=2, w2=3, wna=4, wnb=5, wn2=6

            get_wl(0)  # prefetch: layer 0 starts only ~6us into the kernel

            nt_tiles = [(0, 512), (512, 512), (1024, 512), (1536, 512), (2048, 256)]

            def node_phase_segments(blk, l, hn_src, hn_dst, wA, wB, w_2, bias1, bias2):
                """hn_dst = hn_src + MLP(hn_src, agg); reads hn_src only, writes
                hn_dst (ping-pong) so it runs fully parallel with the edge
                phase. Returned as small segments to interleave between edge
                groups so PE never waits on the intra-phase ACT/DVE chain."""
                state = {}

                def seg_s():
                    # per-graph node sum on the (slack) vector engine, then one
                    # N=256 matmul instead of nine
                    s_raw = sb.tile([H, G], f32r, name=f"sr{blk}_{l}", tag="s_raw", bufs=2)
                    with nc.allow_low_precision(reason="f32r out == matmul rhs rounding"):
                        nc.vector.tensor_reduce(
                            s_raw[:].unsqueeze(2),
                            hn_src[:, 0:S * G].rearrange("p (n g) -> p g n", n=S),
                            mybir.AxisListType.X, ALU.add)
                    ps_s = ps.tile([H, TS], f32, name=f"pss{blk}_{l}", tag="psn", bufs=2)
                    nc.tensor.matmul(ps_s[:, :G], wB, s_raw[:])
                    s_t = sb.tile([H, G], f32r, name=f"st{blk}_{l}", tag="s_t", bufs=2)
                    nc.scalar.activation(s_t[:], ps_s[:, :G], AF.Identity, bias=bias1)
                    state["s_t"] = s_t
                    state["nm"] = []

                def seg_pre(tix):
                    s_t = state["s_t"]
                    for i in tix:
                        off, n = nt_tiles[i]
                        pn = ps.tile([H, TS], f32, name=f"pn{blk}_{l}_{i}", tag="psn", bufs=2)
                        nc.tensor.matmul(pn[:, :n], wA, hn_src[:, off:off + n])
                        # s_t broadcast-add on the (slack) vector engine
                        reps = n // G
                        rhs_s = s_t[:].unsqueeze(1).to_broadcast((H, reps, G))
                        nc.vector.tensor_tensor(
                            pn[:, :n].rearrange("p (a b) -> p a b", a=reps),
                            pn[:, :n].rearrange("p (a b) -> p a b", a=reps),
                            rhs_s, ALU.add)
                        nm = sb.tile([H, TS], f32r, name=f"nm{blk}_{l}_{i}", tag="nm", bufs=5)
                        nc.scalar.activation(nm[:, :n], pn[:, :n], AF.Relu, bias=0.0)
                        state["nm"].append((off, n, nm))

                def seg_post(tix, wrap=False):
                    for i in tix:
                        off, n, nm = state["nm"][i]
                        p2 = ps.tile([H, TS], f32, name=f"pn2{blk}_{l}_{i}", tag="psn", bufs=2)
                        nc.tensor.matmul(p2[:, :n], w_2, nm[:, :n])
                        nc.vector.scalar_tensor_tensor(hn_dst[:, off:off + n], p2[:, :n],
                                                       bias2, hn_src[:, off:off + n],
                                                       ALU.add, ALU.add)
                    if wrap:
                        nc.gpsimd.tensor_copy(hn_dst[:, S * G:17 * G], hn_dst[:, 0:8 * G])

                return [
                    seg_s,
                    lambda: seg_pre([0, 1]),
                    lambda: seg_pre([2, 3]),
                    lambda: seg_pre([4]),
                    lambda: seg_post([0, 1]),
                    lambda: seg_post([2, 3]),
                    lambda: seg_post([4], wrap=True),
                ]

            for blk in range(NBLK):
                he_a = sb.tile([H, ET // 2], f32r, name=f"hea{blk}", tag="he_a")
                he_b = sb.tile([H, ET // 2], f32r, name=f"heb{blk}", tag="he_b")

                def he_sl(t):
                    """he tile-t slice (he is split in halves so block n+1 can
                    recycle each half as soon as the decoder finishes it)."""
                    half, tt = (he_a, t) if t < NTILE // 2 else (he_b, t - NTILE // 2)
                    return half[:, tt * TS:(tt + 1) * TS]
                hn = sb.tile([H, 17 * G], f32r, name=f"hn{blk}", tag="hn", bufs=2)

                # ---------------- node encoder: h_n = x @ enc_n_w + b
                xTb = sb.tile([2, NT], f32r, name=f"xT{blk}", tag="xT_s")
                for off, n in nt_tiles:
                    nc.sync.dma_start(xTb[:, off:off + n],
                                      xT_d.ap()[:, blk * NT + off:blk * NT + off + n])
                for i, (off, n) in enumerate(nt_tiles):
                    pn = ps.tile([H, TS], f32, name=f"ne{blk}_{i}", tag="psn", bufs=2)
                    nc.tensor.matmul(pn[:, :n], encnw, xTb[:, off:off + n])
                    nc.scalar.activation(hn[:, off:off + n], pn[:, :n], AF.Identity, bias=encnb)
                nc.vector.tensor_copy(hn[:, S * G:17 * G], hn[:, 0:8 * G])

                # ----- emission closures (pipelined groups) -----
                ze_map = {}
                msg3_map = {}

                def enc_pre(grp):
                    """Edge encoder group: row-packed K=6 matmuls (4 tiles run
                    concurrently in 4 PE row strips) + relu evicts."""
                    ph = sb.tile([H, TS], f32r, name=f"ph{blk}_{grp}", tag="ph", bufs=3)
                    base = blk * (ET // 4) + grp * TS
                    nc.sync.dma_start(ph[:], physT_d.ap()[:, base:base + TS])
                    pres = []
                    for q in range(4):
                        t = 4 * grp + q
                        tag = "ps1" if q < 3 else "psn"
                        p1 = ps.tile([H, TS], f32, name=f"ee{blk}_{t}", tag=tag, bufs=3 if q < 3 else 2)
                        nc.tensor.matmul(p1[:], encsm[32 * q:32 * q + 6, 0:H],
                                         ph[32 * q:32 * q + 6, :],
                                         tile_position=(32 * q, 0))
                        pres.append((t, p1))
                    cur = []
                    for t, p1 in pres:
                        ze = sb.tile([H, TS], f32r, name=f"ze{blk}_{t}", tag="mz", bufs=18)
                        nc.scalar.activation(ze[:], p1[:], AF.Relu, bias=enceb1)
                        ze_map[t] = ze
                        cur.append((t, ze))
                    return cur

                def edge_pre(l, grp, hn_cur):
                    p1s = []
                    for q in range(GSZ):
                        t = GSZ * grp + q
                        p1 = ps.tile([H, TS], f32, name=f"pe{blk}_{l}_{t}", tag="ps1", bufs=3)
                        p1s.append((t, p1))
                    for t, p1 in p1s:
                        rhs0 = ze_map[t] if l == 0 else he_sl(t)
                        nc.tensor.matmul(p1[:], wsl(l, 0), rhs0,
                                         start=True, stop=False)
                    for t, p1 in p1s:
                        r = t // 4
                        rhs_ta = hn_cur[:, r * G:(r + 1) * G].unsqueeze(1).to_broadcast((H, 2, G))
                        nc.tensor.matmul(p1[:].rearrange("p (a b) -> p a b", a=2),
                                         wsl(l, 1), rhs_ta, start=False, stop=False)
                    for t, p1 in p1s:
                        r, q4 = divmod(t, 4)
                        off = (r + 1 + 2 * q4) * G
                        nc.tensor.matmul(p1[:], wsl(l, 2), hn_cur[:, off:off + TS],
                                         start=False, stop=True)
                    cur = []
                    for t, p1 in p1s:
                        msg = sb.tile([H, TS], f32r, name=f"mg{blk}_{l}_{t}", tag="mz", bufs=18)
                        nc.scalar.activation(msg[:], p1[:], AF.Relu, bias=eb1[:, l:l + 1])
                        if l == 3:
                            msg3_map[t] = msg
                        cur.append((t, msg))
                    return cur

                def edge_w2(l, items):
                    for t, msg in items:
                        p2 = ps.tile([H, TS], f32, name=f"pe2{blk}_{l}_{t}", tag="ps2", bufs=3)
                        if l == 0:
                            # h_e^0 = We2^T ze + be2 is never materialized:
                            # accumulate it here as the residual base instead
                            nc.tensor.matmul(p2[:], encew2, ze_map[t][:],
                                             start=True, stop=False)
                            nc.tensor.matmul(p2[:], wsl(l, 3), msg[:],
                                             start=False, stop=True)
                            nc.vector.tensor_scalar(he_sl(t), p2[:], eb2[:, 0:1],
                                                    None, ALU.add)
                        else:
                            nc.tensor.matmul(p2[:], wsl(l, 3), msg[:])
                            nc.vector.scalar_tensor_tensor(he_sl(t), p2[:], eb2[:, l:l + 1],
                                                           he_sl(t), ALU.add, ALU.add)

                def dec_pre(grp):
                    pr1 = []
                    for q in range(GSZ):
                        t = GSZ * grp + q
                        p1 = ps.tile([H, TS], f32, name=f"pd{blk}_{t}", tag="ps1", bufs=3)
                        nc.tensor.matmul(p1[0:64, :], decw1, he_sl(t),
                                         start=True, stop=False)
                        nc.tensor.matmul(p1[0:64, :], wg[:, 0:64], msg3_map[t][:],
                                         start=False, stop=True)
                        pr1.append((t, p1))
                    cur = []
                    for t, p1 in pr1:
                        z = sb.tile([64, TS], f32r, name=f"z{blk}_{t}", tag="z", bufs=5)
                        nc.scalar.activation(z[:], p1[0:64, :], AF.Relu, bias=decb1x2[0:64, :])
                        cur.append((t, z))
                    return cur

                def dec_tail(items):
                    for i, (t, z) in enumerate(items):
                        tag = "ps2" if i < 3 else "psn"
                        p2 = ps.tile([1, TS], f32, name=f"pd2{blk}_{t}", tag=tag, bufs=3 if i < 3 else 2)
                        nc.tensor.matmul(p2[:], decw2b[0:64, 0:1], z[:])
                        zo = sb.tile([1, TS], f32, name=f"zo{blk}_{t}", tag="zo", bufs=4)
                        nc.vector.tensor_copy(zo[:], p2[:])
                        off = blk * ET + t * TS
                        nc.sync.dma_start(z2_d.ap()[:, off:off + TS], zo[:])

                # ---------------- encoder + layer 0, interleaved.
                # dep math: layer-0 group k reads he tiles 3k..3k+2, which the
                # encoder W2 lag has evicted by combined step k+2.
                hn1 = sb.tile([H, 17 * G], f32r, name=f"hn{blk}_0", tag="hn", bufs=2)
                segs0 = node_phase_segments(blk, 0, hn, hn1,
                                            wsl(0, 4), wsl(0, 5), wsl(0, 6),
                                            nb1[:, 0:1], nb2[:, 0:1])
                l0prev = []
                enc_sched = {0: 0, 1: 1, 3: 2, 5: 3, 7: 4, 8: 5, 9: 6, 10: 7, 12: 8}
                for step in range(NGRP + 3):
                    if step in enc_sched:
                        enc_pre(enc_sched[step])
                    k = step - 2
                    l0cur = edge_pre(0, k, hn) if 0 <= k < NGRP else []
                    edge_w2(0, l0prev)
                    l0prev = l0cur
                    if 1 <= k <= len(segs0):
                        segs0[k - 1]()
                hn_cur = hn1

                # ---------------- layers 1..2 (node segments interleaved)
                for l in (1, 2):
                    hn_next = sb.tile([H, 17 * G], f32r, name=f"hn{blk}_{l}", tag="hn", bufs=2)
                    segs = node_phase_segments(blk, l, hn_cur, hn_next,
                                               wsl(l, 4), wsl(l, 5), wsl(l, 6),
                                               nb1[:, l:l + 1], nb2[:, l:l + 1])
                    prev = []
                    for grp in range(NGRP + 1):
                        cur = edge_pre(l, grp, hn_cur) if grp < NGRP else []
                        edge_w2(l, prev)
                        if 1 <= grp <= len(segs):
                            segs[grp - 1]()
                        prev = cur
                    hn_cur = hn_next

                # ---------------- layer 3 + decoder, interleaved.
                # layer 3 has no node update (its output would be unused).
                # dep math: decoder group k reads he tiles 3k..3k+2, final
                # after layer-3's W2/stt of group k at combined step k+1.
                decprev = []
                for step in range(NGRP + 3):
                    if step < NGRP:
                        edge_pre(3, step, hn_cur)
                    k = step - 2
                    deccur = dec_pre(k) if 0 <= k < NGRP else []
                    dec_tail(decprev)
                    decprev = deccur

    nc.compile()
    return nc


def _get_program():
    if "nc" not in _prog_cache:
        _prog_cache["nc"] = _build_program()
    return _prog_cache["nc"]


# ---------------------------------------------------------------------------
# kernel entry
# ---------------------------------------------------------------------------

def kernel(x_nodes, damage_locs,
           enc_n_w, enc_n_b, enc_e_w1, enc_e_b1, enc_e_w2, enc_e_b2,
           edge_w1, edge_b1, edge_w2, edge_b2,
           node_w1, node_b1, node_w2, node_b2,
           dec_w1, dec_b1, dec_w2, dec_b2,
           edge_index, node_batch):
    import os
    from concourse.bass_utils import run_bass_kernel_spmd

    f32 = np.float32
    x_nodes = np.asarray(x_nodes, f32)
    damage_locs = np.asarray(damage_locs, f32)

    # ---- host precompute
    phys = _build_phys(x_nodes, damage_locs)                  # [B,72,6]

    def cat(ws):
        return np.ascontiguousarray(np.concatenate(ws, axis=0).astype(f32))

    edge_w1 = np.asarray(edge_w1, f32)
    node_w1 = np.asarray(node_w1, f32)
    w1a = cat([edge_w1[l, 0:H, :] for l in range(L)])
    w1b = cat([edge_w1[l, H:2 * H, :] for l in range(L)])
    w1c = cat([edge_w1[l, 2 * H:3 * H, :] for l in range(L)])
    w2 = cat([np.asarray(edge_w2, f32)[l] for l in range(L)])
    wna = cat([node_w1[l, 0:H, :] - node_w1[l, H:2 * H, :] / f32(8.0) for l in range(L)])
    wnb = cat([node_w1[l, H:2 * H, :] / f32(8.0) for l in range(L)])
    wn2 = cat([np.asarray(node_w2, f32)[l] for l in range(L)])
    eb1 = np.ascontiguousarray(np.asarray(edge_b1, f32).T)    # [H,L]
    eb2 = np.ascontiguousarray(np.asarray(edge_b2, f32).T)
    # encoder-We2 fusion into layer 0: pre_0 = (We2 @ W1c0)^T ze + W1c0^T be2 + b1_0
    # and h_e^1 = We2^T ze + be2 + W2_0^T msg + b2_0
    encew2_a = np.asarray(enc_e_w2, f32)
    enceb2_a = np.asarray(enc_e_b2, f32)
    w1c0 = w1c[0:H].copy()
    w1c[0:H] = encew2_a @ w1c0
    eb1[:, 0] = eb1[:, 0] + w1c0.T @ enceb2_a
    eb2[:, 0] = eb2[:, 0] + enceb2_a
    nb1 = np.ascontiguousarray(np.asarray(node_b1, f32).T)
    nb2 = np.ascontiguousarray(np.asarray(node_b2, f32).T)

    dec_w2 = np.asarray(dec_w2, f32)                          # [64, 1]
    decw2b = np.zeros((H, 2), f32)
    decw2b[0:64, 0] = dec_w2[:, 0]
    decw2b[64:128, 1] = dec_w2[:, 0]
    # layer3-W2 + residual fused into dec1: wg = W2_3 @ dec_w1,
    # db1' = dec_b1 + dec_w1^T b2_3
    w2_3 = np.asarray(edge_w2, f32)[3]
    b2_3 = np.asarray(edge_b2, f32)[3]
    decw1_f = np.asarray(dec_w1, f32)
    wg_f = w2_3 @ decw1_f                                     # [H, 64]
    db1p = np.asarray(dec_b1, f32) + decw1_f.T @ b2_3
    decb1x2 = np.concatenate([db1p] * 2)[:, None]

    # packed weights: per layer [w1c | w1a | w1b | w2 | wna | wnb | wn2]
    wl = np.concatenate(
        [np.concatenate([w1c[l * H:(l + 1) * H], w1a[l * H:(l + 1) * H],
                         w1b[l * H:(l + 1) * H], w2[l * H:(l + 1) * H],
                         wna[l * H:(l + 1) * H], wnb[l * H:(l + 1) * H],
                         wn2[l * H:(l + 1) * H]], axis=1) for l in range(L)],
        axis=1)                                               # [H, L*7*H]
    decw1_a = np.asarray(dec_w1, f32)
    wbig = np.concatenate(
        [np.asarray(enc_e_w2, f32), np.eye(H, dtype=f32), decw1_a, decw2b, wg_f], axis=1)
    encsm = np.zeros((H, 2 * H), f32)
    for q in range(4):
        encsm[32 * q:32 * q + 6, 0:H] = np.asarray(enc_e_w1, f32)
    encsm[0:2, H:2 * H] = np.asarray(enc_n_w, f32)
    bpk = np.zeros((H, 20), f32)
    bpk[:, 0:L] = eb1
    bpk[:, L:2 * L] = eb2
    bpk[:, 2 * L:3 * L] = nb1
    bpk[:, 3 * L:4 * L] = nb2
    bpk[:, 16] = np.asarray(enc_n_b, f32)
    bpk[:, 17] = np.asarray(enc_e_b1, f32)
    bpk[:, 18] = np.asarray(enc_e_b2, f32)
    bpk[:, 19] = decb1x2[:, 0]

    shared = dict(
        wl=np.ascontiguousarray(wl),
        wbig=np.ascontiguousarray(wbig),
        encsm=np.ascontiguousarray(encsm),
        bp=np.ascontiguousarray(bpk),
    )

    xg = x_nodes.reshape(B, S, 2)
    in_maps = []
    for c in range(NCORES):
        gsl = slice(c * GC, (c + 1) * GC)
        # xT: [2, blk*NT + n*G + g]
        xc = xg[gsl].reshape(NBLK, G, S, 2).transpose(3, 0, 2, 1).reshape(2, -1)
        # physT: [6, blk*ET + e*G + g] then 4-up row packing:
        # physT4[32q+f, blk*ET/4 + m*TS + j] = pc[f, blk, tile 4m+q, token j]
        pc = phys[gsl].reshape(NBLK, G, EPG, 6).transpose(3, 0, 2, 1).reshape(6, -1)
        pc5 = pc.reshape(6, NBLK, ET // (4 * TS), 4, TS)
        p4 = np.zeros((H, NBLK * ET // 4), f32)
        p4v = p4.reshape(H, NBLK, ET // (4 * TS), TS)
        for q in range(4):
            p4v[32 * q:32 * q + 6] = pc5[:, :, :, q, :]
        m = dict(shared)
        m["xT"] = np.ascontiguousarray(xc)
        m["physT"] = np.ascontiguousarray(p4)
        in_maps.append(m)

    nc = _get_program()
    trace = bool(int(os.environ.get("KERNEL_TRACE", "0")))
    res = None
    for attempt in range(3):
        try:
            res = run_bass_kernel_spmd(nc, in_maps, core_ids=list(range(NCORES)),
                                       trace=trace)
            break
        except Exception:
            if attempt == 2:
                raise
    _prog_cache["last_results"] = res

    # ---- host postprocess: sigmoid + pair mean
    z2 = np.empty((B, EPG), f32)
    for c in range(NCORES):
        zc = res.results[c]["z2"].reshape(NBLK, EPG, G).transpose(0, 2, 1).reshape(GC, EPG)
        z2[c * GC:(c + 1) * GC] = zc

    logits = z2 + np.asarray(dec_b2, f32)[0]
    sig = f32(1.0) / (f32(1.0) + np.exp(-logits))

    pairs = [(i, j) for i in range(S) for j in range(i + 1, S)]
    out = np.empty((B, len(pairs)), f32)
    for p, (i, j) in enumerate(pairs):
        a = i * 8 + (j - i - 1)
        bidx = j * 8 + (8 - (j - i))
        out[:, p] = f32(0.5) * (sig[:, a] + sig[:, bidx])
    return out



# revision 21
# speedup vs baseline: 1.1691x; 1.1691x over previous
"""DirectPathAttenuationGNN Trainium2 kernel — fp8 DoubleRow delta-stream design.

Data-parallel over graphs (512 graphs/core x 8 cores), K9 topology hardcoded.
Per core: 2 blocks x 256 graphs; per block 18432 edge tokens (72 edge slots x
256 graphs), 2304 node tokens.

Compute design:
- All heavy per-edge matmuls run as fp8(e4m3) DoubleRow (0.5 cy/col), with the
  two k-planes used for algebraic pairing:
    p1_l   = (zef_l | w1cd_l) @ (ze | d0) [+ (w1cd_l | w1cd_l)@(d1 | d2)]
             + (w1a | w1b) @ (hn_r | hn_c)   per 256-token half
    d_l    = (w2_hi | w2_lo) @ (msg_l | msg_l)      (weight hi/lo split)
    z      = (zdec | ddec)@(ze|d0) + (ddec|ddec)@(d1|d2) + (wg_hi|wg_lo)@msg3
- Residual stream is kept as DELTAS (he_l = he0 + sum_m d_m, he0 = encew2@ze
  folded into the ze planes) so the running state is never re-quantized.
- All psum group scales equal the output fp8 scale, so evictions are single
  ACT (relu/copy + bias) or DVE (tensor_scalar add-bias/max0) ops, greedily
  balanced between the two engines.
- Logits: 12 tiles accumulate into distinct rows of a [16,512] psum via
  per-tile bf16 weight columns; sigmoid + pair-mean on host.
- Node path (hn trajectory) stays f32r; hn is re-quantized to a wrapped fp8
  copy each layer for the edge gathers.
"""

import sys

if "/opt/trn_rl_repo" not in sys.path:
    sys.path.insert(0, "/opt/trn_rl_repo")

import numpy as np
import ml_dtypes

B = 4096
S = 9
EPG = 72
H = 128
L = 4
NCORES = 8
GC = B // NCORES          # graphs per core = 512
G = 256                   # graphs per block
NBLK = GC // G            # 2
ET = EPG * G              # 18432 edge tokens per block
NT = S * G                # 2304 node tokens per block
TILE = 512                # tokens per tile (2 edge slots)
NTILE = ET // TILE        # 36
GRP = 1024                # tokens per psum/eviction group (2 tiles)
NGRP = ET // GRP          # 18
PHYSC = 2048              # phys chunk tokens (9 chunks per block)
EPS = np.float32(1e-8)

FP8 = ml_dtypes.float8_e4m3
BF = ml_dtypes.bfloat16

# pow2 scales (calibrated offline on the reference input distribution)
SPH = 16.0   # phys
SZE = 8.0    # ze
SMSG = 16.0  # msg (all layers)
SD = 16.0    # deltas d0..d2
SHN = 8.0    # hn fp8 copies
SGZ = 16.0   # z group scale (z stored bf16 as SGZ*z)

# arena offsets (fp8 bytes per partition)
ZE_O = 0
D_O = [ET, 2 * ET, 3 * ET]
MSG_O = 4 * ET
MSG_SLOTS = 8
HN8_O = 4 * ET + MSG_SLOTS * GRP
HN8_W = 17 * G            # wrapped hn
AR_W = HN8_O + 2 * HN8_W

_prog_cache = {}


def _edge_struct():
    r_idx = np.repeat(np.arange(S), 8)
    k_idx = np.tile(np.arange(8), S)
    c_idx = (r_idx + 1 + k_idx) % S
    return r_idx, c_idx


def _build_phys(x_nodes, damage_locs):
    xg = x_nodes.reshape(B, S, 2)
    r_idx, c_idx = _edge_struct()
    src = xg[:, r_idx, :]
    dst = xg[:, c_idx, :]
    dmg = damage_locs[:, None, :]
    vec = src - dst
    edge_len = np.sqrt(np.sum(vec * vec, -1) + EPS)
    d21 = dst - src
    l2 = np.clip(np.sum(d21 * d21, -1), EPS, None)
    t = np.clip(np.sum((dmg - src) * d21, -1) / l2, np.float32(0.0), np.float32(1.0))
    proj = src + t[..., None] * d21
    d_path = np.sqrt(np.sum((dmg - proj) ** 2, -1) + EPS)
    d_tx = np.sqrt(np.sum((src - dmg) ** 2, -1) + EPS)
    d_rx = np.sqrt(np.sum((dst - dmg) ** 2, -1) + EPS)
    phys = np.stack([vec[..., 0], vec[..., 1], edge_len, d_path, d_tx, d_rx], -1)
    return np.ascontiguousarray(phys.astype(np.float32))


# weight pack column layout (fp8 [128, WP_W]); entries are [K, 2, M] pairs
def _wpack_layout():
    off = 0
    lay = {}

    def add(name, m2):
        nonlocal off
        lay[name] = off
        off += m2

    add("enc", 256)
    for l in range(L):
        add(f"p1a_{l}", 256)          # (zef_l | w1cd_l or 0)
    add("p1b_2", 256)                 # (w1cd_2 | 0)
    add("p1b_3", 256)                 # (w1cd_3 | w1cd_3)
    for l in range(L):
        add(f"node_{l}", 256)         # (w1a'_l | w1b'_l)
    for l in range(3):
        add(f"w2_{l}", 256)           # (w2_hi | w2_lo)
    add("deca", 128)                  # (zdec | ddec)
    add("decb", 128)                  # (ddec | ddec)
    add("wg", 128)                    # (wg_hi | wg_lo)
    return lay, off


WLAY, WP_W = _wpack_layout()

# node f32 weight pack [128, NW_W]
def _nodew_layout():
    off = 0
    lay = {}
    for l in range(3):
        for nm in ("wna", "wnb", "wn2"):
            lay[f"{nm}_{l}"] = off
            off += H
    lay["encn"] = off
    off += H
    return lay, off


NLAY, NW_W = _nodew_layout()

NBC = 16  # bias columns: 0-3 b1s_l, 4 zbias, 5-7 nb1_l, 8-10 nb2_l, 11 encnb


def _build_program():
    from concourse import bacc, mybir, tile
    from concourse.ap import AP
    from contextlib import ExitStack

    f32 = mybir.dt.float32
    f32r = mybir.dt.float32r
    bf16 = mybir.dt.bfloat16
    f8 = mybir.dt.float8e4
    AF = mybir.ActivationFunctionType
    ALU = mybir.AluOpType
    DRM = mybir.MatmulPerfMode.DoubleRow

    nc = bacc.Bacc("TRN2", target_bir_lowering=False, debug=False)

    physP_d = nc.dram_tensor("physP", [6, NBLK * 2 * ET], f8, kind="ExternalInput")
    xT_d = nc.dram_tensor("xT", [2, NBLK * NT], f32r, kind="ExternalInput")
    wp_d = nc.dram_tensor("wp", [H, WP_W], f8, kind="ExternalInput")
    wz_d = nc.dram_tensor("wz", [64, 12 * 16], bf16, kind="ExternalInput")
    nw_d = nc.dram_tensor("nw", [H, NW_W], f32r, kind="ExternalInput")
    bs_d = nc.dram_tensor("bs", [H, NBC], f32, kind="ExternalInput")
    z2_d = nc.dram_tensor("z2", [16, NBLK * 3 * TILE], f32, kind="ExternalOutput")

    # static engine-balance accounting (ns-ish units) for eviction assignment
    bal = {"A": 0.0, "D": 0.0}

    def pick_engine(cost_a, cost_d):
        if bal["A"] + cost_a <= bal["D"] + cost_d:
            bal["A"] += cost_a
            return "A"
        bal["D"] += cost_d
        return "D"

    with nc.allow_low_precision(reason="fp8 kernel by design"):
        with tile.TileContext(nc) as tc:
            with ExitStack() as ctx:
                wpool = ctx.enter_context(tc.tile_pool(name="w", bufs=1))
                sb = ctx.enter_context(tc.tile_pool(name="sb", bufs=1))
                ps = ctx.enter_context(tc.tile_pool(name="ps", bufs=1, space="PSUM"))

                wp = wpool.tile([H, WP_W], f8, name="wp", tag="wp")
                nc.sync.dma_start(wp[:], wp_d.ap())
                bs = wpool.tile([H, NBC], f32, name="bs", tag="bs")
                nc.sync.dma_start(bs[:], bs_d.ap())
                nw = wpool.tile([H, NW_W], f32r, name="nw", tag="nw")
                nc.sync.dma_start(nw[:], nw_d.ap())
                wz = wpool.tile([64, 12 * 16], bf16, name="wz", tag="wz")
                nc.sync.dma_start(wz[:], wz_d.ap())

                def wsl(name, m=128, k=H):
                    c = WLAY[name]
                    return wp[0:k, c:c + 2 * m].rearrange("k (two m) -> k two m", two=2)

                def nsl(name):
                    c = NLAY[name]
                    return nw[:, c:c + H]

                arena = sb.tile([H, AR_W], f8, name="arena", tag="arena")
                ARP = [AR_W, H]  # partition dim of arena APs

                def rp(off0, off1, n):
                    """rhs [128, 2, n] with planes at arena offsets off0/off1"""
                    return AP(arena.tensor, off0,
                              [list(ARP), [off1 - off0, 2], [1, n]])

                # z slots (bf16) and logit staging
                zslots = sb.tile([64, 4 * GRP], bf16, name="zslots", tag="zslots")
                zo = sb.tile([16, NBLK * 3 * TILE], f32, name="zo", tag="zo")
                # node f32r state
                hnT = [sb.tile([H, NT], f32r, name=f"hn{i}", tag=f"hn{i}")
                       for i in range(2)]

                nt_tiles = [(0, 512), (512, 512), (1024, 512), (1536, 512), (2048, 256)]

                gcnt = [0]  # global group counter (msg slot rotation)

                def evict(kind, psrc, dst_ap, bias_ap=None, scale=1.0, n=GRP,
                          eng=None):
                    """psum->sbuf eviction; eng forces 'A'/'D', else greedy."""
                    ca = n * 0.8333 + 185.0
                    cd = n * 1.0417 + 130.0
                    if eng is None:
                        e = pick_engine(ca, cd)
                    else:
                        e = eng
                        bal["A" if e == "A" else "D"] += ca if e == "A" else cd
                    if kind == "relu":
                        if e == "A":
                            nc.scalar.activation(dst_ap, psrc, AF.Relu,
                                                 bias=bias_ap, scale=scale)
                        else:
                            nc.vector.tensor_scalar(dst_ap, psrc, bias_ap, 0.0,
                                                    ALU.add, ALU.max)
                    else:
                        if e == "A":
                            nc.scalar.activation(dst_ap, psrc, AF.Copy,
                                                 bias=0.0, scale=scale)
                        else:
                            if scale == 1.0:
                                nc.vector.tensor_copy(dst_ap, psrc)
                            else:
                                nc.vector.tensor_scalar(dst_ap, psrc, scale, None,
                                                        ALU.mult)

                # ---------------- node phase -----------------
                def node_phase_segments(blk, l, src_i, dst_i):
                    """hn_{l+1} = hn_l + wn2@relu(wna@hn + wnb@s_bcast + nb1) + nb2,
                    then produce wrapped fp8 copy of hn_{l+1}. Returns segments."""
                    hsrc, hdst = hnT[src_i], hnT[dst_i]
                    st = {}

                    def seg_s():
                        sr = sb.tile([H, G], f32r, name=f"sr{blk}_{l}", tag="sraw", bufs=2)
                        nc.vector.tensor_reduce(
                            sr[:].unsqueeze(2),
                            hsrc[:].rearrange("p (n g) -> p g n", n=S),
                            mybir.AxisListType.X, ALU.add)
                        bal["D"] += NT * 1.0417 + 130
                        st["sr"] = sr
                        st["nm"] = []

                    def seg_pre(tix):
                        for i in tix:
                            off, n = nt_tiles[i]
                            pn = ps.tile([H, 512], f32, name=f"pn{blk}_{l}_{i}",
                                         tag="small", bufs=2)
                            nc.tensor.matmul(pn[:, :n], nsl(f"wna_{l}"),
                                             hsrc[:, off:off + n],
                                             start=True, stop=False)
                            reps = n // G
                            rhs_s = st["sr"][:].unsqueeze(1).to_broadcast((H, reps, G))
                            nc.tensor.matmul(pn[:, :n].rearrange("p (a b) -> p a b", a=reps),
                                             nsl(f"wnb_{l}"), rhs_s,
                                             start=False, stop=True,
                                             skip_group_check=True)
                            nm = sb.tile([H, 512], f32r, name=f"nm{blk}_{l}_{i}",
                                         tag="nm", bufs=5)
                            nc.scalar.activation(nm[:, :n], pn[:, :n], AF.Relu,
                                                 bias=bs[:, 5 + l:6 + l])
                            bal["A"] += n * 0.8333 + 185
                            st["nm"].append((off, n, nm))

                    def seg_post(tix):
                        for i in tix:
                            off, n, nm = st["nm"][i]
                            p2 = ps.tile([H, 512], f32, name=f"pn2{blk}_{l}_{i}",
                                         tag="small", bufs=2)
                            nc.tensor.matmul(p2[:, :n], nsl(f"wn2_{l}"), nm[:, :n])
                            nc.vector.scalar_tensor_tensor(
                                hdst[:, off:off + n], p2[:, :n], bs[:, 8 + l:9 + l],
                                hsrc[:, off:off + n], ALU.add, ALU.add)
                            bal["D"] += n * 1.0417 + 130

                    def seg_q(part):
                        # fp8 wrapped copy of hn_{l+1} (for layer l+1 edge gathers)
                        slot = HN8_O + ((l + 1) % 2) * HN8_W
                        if part == 0:
                            nc.gpsimd.tensor_scalar(
                                AP(arena.tensor, slot, [list(ARP), [1, 9 * G]]),
                                hdst[:], SHN, None, ALU.mult)
                        else:
                            nc.gpsimd.tensor_copy(
                                AP(arena.tensor, slot + 9 * G, [list(ARP), [1, 8 * G]]),
                                AP(arena.tensor, slot, [list(ARP), [1, 8 * G]]))

                    return [seg_s,
                            lambda: seg_pre([0, 1]),
                            lambda: seg_pre([2, 3]),
                            lambda: seg_pre([4]),
                            lambda: seg_post([0, 1]),
                            lambda: seg_post([2, 3]),
                            lambda: seg_post([4]),
                            lambda: seg_q(0),
                            lambda: seg_q(1)]

                def node_encoder(blk):
                    xTb = sb.tile([2, NT], f32r, name=f"xTb{blk}", tag="xT", bufs=2)
                    nc.sync.dma_start(xTb[:], xT_d.ap()[:, blk * NT:(blk + 1) * NT])
                    for i, (off, n) in enumerate(nt_tiles):
                        pn = ps.tile([H, 512], f32, name=f"ne{blk}_{i}",
                                     tag="small", bufs=2)
                        nc.tensor.matmul(pn[:, :n], nsl("encn")[0:2, :],
                                         xTb[:, off:off + n])
                        nc.scalar.activation(hnT[0][:, off:off + n], pn[:, :n],
                                             AF.Identity, bias=bs[:, 11:12])
                    slot = HN8_O + 0 * HN8_W
                    nc.gpsimd.tensor_scalar(
                        AP(arena.tensor, slot, [list(ARP), [1, 9 * G]]),
                        hnT[0][:], SHN, None, ALU.mult)
                    nc.gpsimd.tensor_copy(
                        AP(arena.tensor, slot + 9 * G, [list(ARP), [1, 8 * G]]),
                        AP(arena.tensor, slot, [list(ARP), [1, 8 * G]]))

                # ---------------- edge phases -----------------
                def enc_group(blk, g, pchunks):
                    ci = (g * GRP) // PHYSC
                    pc = pchunks[ci]
                    pz = ps.tile([H, GRP], f32, name=f"pz{blk}_{g}", tag="p1", bufs=2)
                    for ti in range(2):
                        tok = g * GRP + ti * TILE
                        coff = tok - ci * PHYSC
                        rhs = AP(pc.tensor, coff,
                                 [[2 * PHYSC, 6], [PHYSC, 2], [1, TILE]])
                        nc.tensor.matmul(pz[:, ti * TILE:(ti + 1) * TILE],
                                         wsl("enc", k=6), rhs, perf_mode=DRM)
                    evict("relu", pz[:],
                          AP(arena.tensor, ZE_O + g * GRP, [list(ARP), [1, GRP]]),
                          bias_ap=bs[:, 12:13], eng=("A" if g % 2 else "D"))

                def p1_group(blk, l, g, hn8_slot):
                    p1 = ps.tile([H, GRP], f32, name=f"p1_{blk}_{l}_{g}",
                                 tag="p1", bufs=2)
                    for ti in range(2):
                        t = 2 * g + ti
                        tok = t * TILE
                        bank = p1[:, ti * TILE:(ti + 1) * TILE]
                        # plane pair (ze, d0) — L0 pairs (ze, ze) with zero plane1
                        o1 = (D_O[0] + tok) if l > 0 else (ZE_O + tok)
                        nc.tensor.matmul(bank, wsl(f"p1a_{l}"),
                                         rp(ZE_O + tok, o1, TILE),
                                         perf_mode=DRM, start=True, stop=False)
                        if l >= 2:
                            o2 = (D_O[2] + tok) if l == 3 else (D_O[1] + tok)
                            nc.tensor.matmul(bank, wsl(f"p1b_{l}"),
                                             rp(D_O[1] + tok, o2, TILE),
                                             perf_mode=DRM, start=False, stop=False,
                                             skip_group_check=True)
                        r, q = t // 4, t % 4
                        for hh in range(2):
                            ro = hn8_slot + r * G
                            co = hn8_slot + (r + 1 + 2 * q + hh) * G
                            nc.tensor.matmul(bank[:, hh * G:(hh + 1) * G],
                                             wsl(f"node_{l}"), rp(ro, co, G),
                                             perf_mode=DRM, start=False,
                                             stop=(hh == 1), skip_group_check=True)
                    mslot = MSG_O + (gcnt[0] % MSG_SLOTS) * GRP
                    gcnt[0] += 1
                    evict("relu", p1[:],
                          AP(arena.tensor, mslot, [list(ARP), [1, GRP]]),
                          bias_ap=bs[:, l:l + 1], eng=("A" if g % 2 else "D"))
                    return mslot

                def d_group(blk, l, g, mslot):
                    for ti in range(2):
                        pd = ps.tile([H, TILE], f32, name=f"pd{blk}_{l}_{g}_{ti}",
                                     tag="pd", bufs=2)
                        nc.tensor.matmul(pd[:],
                                         wsl(f"w2_{l}"),
                                         rp(mslot + ti * TILE, mslot + ti * TILE, TILE),
                                         perf_mode=DRM)
                        evict("copy", pd[:],
                              AP(arena.tensor, D_O[l] + g * GRP + ti * TILE,
                                 [list(ARP), [1, TILE]]),
                              n=TILE)

                def dec_group(blk, g, mslot, lp_state):
                    pzz = ps.tile([64, GRP], f32, name=f"pzz{blk}_{g}",
                                  tag="p1", bufs=2)
                    for ti in range(2):
                        tok = (2 * g + ti) * TILE
                        bank = pzz[:, ti * TILE:(ti + 1) * TILE]
                        nc.tensor.matmul(bank, wsl("deca", m=64),
                                         rp(ZE_O + tok, D_O[0] + tok, TILE),
                                         perf_mode=DRM, start=True, stop=False)
                        nc.tensor.matmul(bank, wsl("decb", m=64),
                                         rp(D_O[1] + tok, D_O[2] + tok, TILE),
                                         perf_mode=DRM, start=False, stop=False,
                                         skip_group_check=True)
                        nc.tensor.matmul(bank, wsl("wg", m=64),
                                         rp(mslot + ti * TILE, mslot + ti * TILE, TILE),
                                         perf_mode=DRM, start=False, stop=True,
                                         skip_group_check=True)
                    zslot = zslots[:, (g % 4) * GRP:(g % 4 + 1) * GRP]
                    evict("relu", pzz[:], zslot, bias_ap=bs[0:64, 4:5],
                          eng=("A" if g % 2 else "D"))
                    # dec2: per tile into logit psum rows
                    for ti in range(2):
                        t = 2 * g + ti
                        gi, i = t // 12, t % 12
                        if i == 0:
                            lp_state["pl"] = ps.tile([16, TILE], f32,
                                                     name=f"pl{blk}_{gi}",
                                                     tag="small", bufs=2)
                        nc.tensor.matmul(lp_state["pl"][:],
                                         wz[:, i * 16:(i + 1) * 16],
                                         zslot[:, ti * TILE:(ti + 1) * TILE],
                                         start=(i == 0), stop=(i == 11),
                                         skip_group_check=True)
                        if i == 11:
                            dst = zo[:, (blk * 3 + gi) * TILE:(blk * 3 + gi + 1) * TILE]
                            nc.vector.tensor_copy(dst, lp_state["pl"][:])
                            bal["D"] += TILE * 1.0417 + 130

                # ================= main schedule =================
                for blk in range(NBLK):
                    node_encoder(blk)
                    # phys chunks for this block
                    pchunks = []
                    for ci in range(ET // PHYSC):
                        t8 = sb.tile([6, 2 * PHYSC], f8, name=f"ph{blk}_{ci}",
                                     tag="ph", bufs=4)
                        src = AP(physP_d.ap().tensor, blk * 2 * ET + ci * PHYSC,
                                 [[NBLK * 2 * ET, 6], [ET, 2], [1, PHYSC]])
                        dst = t8[:].rearrange("p (a b) -> p a b", a=2)
                        nc.sync.dma_start(dst, src)
                        pchunks.append(t8)

                    for g in range(NGRP):
                        enc_group(blk, g, pchunks)

                    segs = {0: node_phase_segments(blk, 0, 0, 1),
                            1: node_phase_segments(blk, 1, 1, 0),
                            2: node_phase_segments(blk, 2, 0, 1)}
                    for l in range(3):
                        hn8_slot = HN8_O + (l % 2) * HN8_W
                        sl = segs[l]
                        for g in range(NGRP):
                            mslot = p1_group(blk, l, g, hn8_slot)
                            d_group(blk, l, g, mslot)
                            si = (g * len(sl)) // NGRP
                            si2 = ((g + 1) * len(sl)) // NGRP
                            for k in range(si, si2):
                                sl[k]()
                    # L3 + dec interleaved (dec lags by 2 groups)
                    hn8_slot = HN8_O + (3 % 2) * HN8_W
                    lp_state = {}
                    mslots = {}
                    for g in range(NGRP + 2):
                        if g < NGRP:
                            mslots[g] = p1_group(blk, 3, g, hn8_slot)
                        k = g - 2
                        if 0 <= k < NGRP:
                            dec_group(blk, k, mslots.pop(k), lp_state)
                    nc.sync.dma_start(
                        z2_d.ap()[:, blk * 3 * TILE:(blk + 1) * 3 * TILE],
                        zo[:, blk * 3 * TILE:(blk + 1) * 3 * TILE])

    nc.compile()
    return nc


def _get_program():
    if "nc" not in _prog_cache:
        _prog_cache["nc"] = _build_program()
    return _prog_cache["nc"]


# ---------------------------------------------------------------------------
# host-side quantization helpers
# ---------------------------------------------------------------------------

def _q8(x):
    return np.asarray(x, FP8)


def _hilo(x):
    hi = _q8(x)
    lo = _q8(x - hi.astype(np.float32))
    return hi, lo


# ---------------------------------------------------------------------------
# kernel entry
# ---------------------------------------------------------------------------

def kernel(x_nodes, damage_locs,
           enc_n_w, enc_n_b, enc_e_w1, enc_e_b1, enc_e_w2, enc_e_b2,
           edge_w1, edge_b1, edge_w2, edge_b2,
           node_w1, node_b1, node_w2, node_b2,
           dec_w1, dec_b1, dec_w2, dec_b2,
           edge_index, node_batch):
    import os
    from concourse.bass_utils import run_bass_kernel_spmd

    f32 = np.float32
    x_nodes = np.asarray(x_nodes, f32)
    damage_locs = np.asarray(damage_locs, f32)
    encew2 = np.asarray(enc_e_w2, f32)
    edge_w1 = np.asarray(edge_w1, f32)
    edge_w2a = np.asarray(edge_w2, f32)
    dec_w1a = np.asarray(dec_w1, f32)

    phys = _build_phys(x_nodes, damage_locs)                  # [B,72,6]

    # ---- weight pack (fp8)
    wp = np.zeros((H, WP_W), FP8)

    def put_pair(name, p0, p1, k=H, m=128):
        c = WLAY[name]
        wp[0:k, c:c + m] = p0
        wp[0:k, c + m:c + 2 * m] = p1

    wencp = np.asarray(enc_e_w1, f32) * f32(SZE / SPH)
    put_pair("enc", _q8(wencp), _q8(wencp), k=6)
    zero128 = np.zeros((H, H), FP8)
    for l in range(L):
        w1c = edge_w1[l, 2 * H:3 * H, :]
        zef = _q8((encew2 @ w1c) * f32(SMSG / SZE))
        w1cd = _q8(w1c * f32(SMSG / SD))
        put_pair(f"p1a_{l}", zef, (zero128 if l == 0 else w1cd))
        w1a = _q8(edge_w1[l, 0:H, :] * f32(SMSG / SHN))
        w1b = _q8(edge_w1[l, H:2 * H, :] * f32(SMSG / SHN))
        put_pair(f"node_{l}", w1a, w1b)
        if l >= 2:
            put_pair(f"p1b_{l}", w1cd, (zero128 if l == 2 else w1cd))
    for l in range(3):
        hi, lo = _hilo(edge_w2a[l] * f32(SD / SMSG))
        put_pair(f"w2_{l}", hi, lo)
    zdec = _q8((encew2 @ dec_w1a) * f32(SGZ / SZE))
    ddec = _q8(dec_w1a * f32(SGZ / SD))
    put_pair("deca", zdec, ddec, m=64)
    put_pair("decb", ddec, ddec, m=64)
    wg = edge_w2a[3] @ dec_w1a
    hi, lo = _hilo(wg * f32(SGZ / SMSG))
    put_pair("wg", hi, lo, m=64)

    # dec2 variants (bf16): w2dec/SGZ at column i
    wzv = np.zeros((64, 12, 16), f32)
    for i in range(12):
        wzv[:, i, i % 16] = np.asarray(dec_w2, f32)[:, 0] / f32(SGZ)
    wz = np.asarray(wzv.reshape(64, 192), BF)

    # node f32 weights
    nwp = np.zeros((H, NW_W), f32)
    node_w1a = np.asarray(node_w1, f32)
    for l in range(3):
        nwp[:, NLAY[f"wna_{l}"]:NLAY[f"wna_{l}"] + H] = (
            node_w1a[l, 0:H, :] - node_w1a[l, H:2 * H, :] / f32(8.0))
        nwp[:, NLAY[f"wnb_{l}"]:NLAY[f"wnb_{l}"] + H] = node_w1a[l, H:2 * H, :] / f32(8.0)
        nwp[:, NLAY[f"wn2_{l}"]:NLAY[f"wn2_{l}"] + H] = np.asarray(node_w2, f32)[l]
    nwp[0:2, NLAY["encn"]:NLAY["encn"] + H] = np.asarray(enc_n_w, f32)

    # biases
    bsc = np.zeros((H, NBC), f32)
    eb2sum = np.asarray(enc_e_b2, f32).copy()
    for l in range(L):
        w1c = edge_w1[l, 2 * H:3 * H, :]
        b1f = np.asarray(edge_b1, f32)[l] + w1c.T @ eb2sum
        bsc[:, l] = b1f * f32(SMSG)
        eb2sum = eb2sum + np.asarray(edge_b2, f32)[l]
    db1 = np.asarray(dec_b1, f32) + dec_w1a.T @ eb2sum                  # incl b2_3
    bsc[0:64, 4] = db1 * f32(SGZ)
    for l in range(3):
        bsc[:, 5 + l] = np.asarray(node_b1, f32)[l]
        bsc[:, 8 + l] = np.asarray(node_b2, f32)[l]
    bsc[:, 11] = np.asarray(enc_n_b, f32)
    bsc[:, 12] = np.asarray(enc_e_b1, f32) * f32(SZE)

    # ---- per-core inputs
    shared = dict(wp=wp, wz=wz, nw=np.ascontiguousarray(nwp),
                  bs=np.ascontiguousarray(bsc))
    xg = x_nodes.reshape(B, S, 2)
    in_maps = []
    for c in range(NCORES):
        gsl = slice(c * GC, (c + 1) * GC)
        xc = xg[gsl].reshape(NBLK, G, S, 2).transpose(3, 0, 2, 1).reshape(2, -1)
        pc = phys[gsl].reshape(NBLK, G, EPG, 6).transpose(3, 0, 2, 1)  # [6,NBLK,72,G]
        pc = pc.reshape(6, NBLK, ET) * f32(SPH)
        hi = _q8(pc)
        lo = _q8(pc - hi.astype(f32))
        pp = np.zeros((6, NBLK, 2, ET), FP8)
        pp[:, :, 0, :] = hi
        pp[:, :, 1, :] = lo
        m = dict(shared)
        m["physP"] = np.ascontiguousarray(pp.reshape(6, NBLK * 2 * ET))
        m["xT"] = np.ascontiguousarray(xc)
        in_maps.append(m)

    nc = _get_program()
    trace = bool(int(os.environ.get("KERNEL_TRACE", "0")))
    res = None
    for attempt in range(3):
        try:
            res = run_bass_kernel_spmd(nc, in_maps, core_ids=list(range(NCORES)),
                                       trace=trace)
            break
        except Exception:
            if attempt == 2:
                raise
    _prog_cache["last_results"] = res

    # ---- host postprocess
    db2 = np.asarray(dec_b2, f32)[0]
    out = np.empty((B, 36), f32)
    pairs = [(i, j) for i in range(S) for j in range(i + 1, S)]
    for c in range(NCORES):
        zr = res.results[c]["z2"]                 # [16, NBLK*3*512]
        logit = np.empty((NBLK, EPG, G), f32)
        zr4 = zr.reshape(16, NBLK, 3, TILE)
        for b in range(NBLK):
            for gi in range(3):
                for i in range(12):
                    t = gi * 12 + i
                    seg = zr4[i, b, gi]                       # [512]
                    logit[b, 2 * t, :] = seg[0:G]
                    logit[b, 2 * t + 1, :] = seg[G:2 * G]
        sig = f32(1.0) / (f32(1.0) + np.exp(-(logit + db2)))  # [NBLK,72,G]
        sig = sig.transpose(0, 2, 1).reshape(GC, EPG)         # [GC, 72]
        for p, (i, j) in enumerate(pairs):
            a = i * 8 + (j - i - 1)
            bidx = j * 8 + (8 - (j - i))
            out[c * GC:(c + 1) * GC, p] = f32(0.5) * (sig[:, a] + sig[:, bidx])
    return out
